# revision 2
# baseline (speedup 1.0000x reference)
"""GCN layer (Linear -> weighted-adjacency SpMM -> BatchNorm(eval) -> exact GELU)
as a Bass/Tile kernel on 8 Trainium2 NeuronCores.

Sharding: both source and destination nodes are sharded across the 8 cores
(12500 rows each).  Phase 1 computes the LOCAL shard of
`support = x @ W' + b'` only (x arrives pre-sharded + transposed + bf16 per
core, 13MB instead of a replicated 205MB f32 copy), written as 4 row-quarters.
Each quarter is AllGathered across cores into a section tensor
`sec[q] = [core0.q | core1.q | ... | core7.q]` (25600 rows, int16-addressable),
so phase-2 gathers of section q only depend on quarter-q collectives and
overlap with the rest of phase 1 + other section passes.

Phase 2 per destination tile (128 rows): source rows of the gathered section
are fetched with one big `dma_gather` (int16 section-local indices, runtime
valid-count register, negative-index tail padding), and segment-summed on the
tensor engine with per-128-edge-chunk one-hot selector matmuls accumulating in
PSUM; partial sums per section accumulate into an SBUF-resident f32
accumulator.  BN is folded on the host (W' = W * s, shift = beta - mean * s,
s = gamma / sqrt(var + eps)), so the epilogue is one add + one exact-GELU
activation per tile.  All feature data (x, W, support, gathers, selector) is
bf16 — halves both the PCIe input streaming and the random-gather HBM traffic
that dominate this memory-regime problem; accumulation stays f32 in PSUM/SBUF.

Host-side prep inside kernel(): shard + transpose + bf16-cast x; per core
group edges by (source-quarter, destination-tile); pack per-group edge row/val
into fixed 128-edge chunk layout and indices into the 16-partition-wrapped
int16 layout dma_gather expects (replicated 16->128 partitions on device, not
on the host, to save PCIe bytes).  One SPMD program serves all 8 cores;
per-group edge counts ride in as data (the count register trims descriptor
generation to real edges).
"""

import sys

sys.path.insert(0, "/opt/trn_rl_repo")

import numpy as np

import concourse.tile as tile
from concourse import bacc, mybir
from concourse.bass_utils import run_bass_kernel_spmd

F32 = mybir.dt.float32
BF16 = mybir.dt.bfloat16
I32 = mybir.dt.int32
I16 = mybir.dt.int16
AF = mybir.ActivationFunctionType
ALU = mybir.AluOpType
NPBF16 = mybir.dt.np(mybir.dt.bfloat16)

N_CORES = 8
SHARD = 12500   # valid rows per core
SH = 12800      # padded rows per core (100 tiles)
QROWS = 3200    # rows per quarter (SH/4), 25 tiles
TPS = 14        # dest tiles per index-slab load (must divide nt)
XCOLS = 640     # node columns per phase-1 supertile (divides QROWS)
NGBUF = 5       # round-robin gather buffers


def _build_program(*, in_dim, out_dim, nt, c_sub, tps, xcols):
    assert in_dim % 128 == 0 and QROWS % xcols == 0
    assert nt % tps == 0
    kb = in_dim // 128
    sec_rows = QROWS * N_CORES          # 25600, int16-addressable
    nsup_q = QROWS // xcols             # supertiles per quarter
    jt = xcols // 128
    nidx = c_sub * 128
    idxcols = nidx // 16
    ng = 4 * nt

    nc = bacc.Bacc("TRN2", target_bir_lowering=False, debug=False,
                   num_devices=N_CORES)

    xT = nc.dram_tensor("xT", [in_dim, SH], BF16, kind="ExternalInput").ap()
    Wp = nc.dram_tensor("Wp", [in_dim, out_dim], BF16, kind="ExternalInput").ap()
    bp = nc.dram_tensor("bp", [128, out_dim], F32, kind="ExternalInput").ap()
    shiftb = nc.dram_tensor("shiftb", [128, out_dim], F32, kind="ExternalInput").ap()
    iota_in = nc.dram_tensor("iota", [128, 128], BF16, kind="ExternalInput").ap()
    idx16 = nc.dram_tensor("idx16", [16, ng * idxcols], I16,
                           kind="ExternalInput").ap()
    rowp = nc.dram_tensor("rowp", [128, ng * c_sub], BF16,
                          kind="ExternalInput").ap()
    valp = nc.dram_tensor("valp", [128, ng * c_sub], BF16,
                          kind="ExternalInput").ap()
    cnts = nc.dram_tensor("cnts", [1, ng], I32, kind="ExternalInput").ap()
    out = nc.dram_tensor("out", [nt * 128, out_dim], BF16,
                         kind="ExternalOutput").ap()
    sup_local = nc.dram_tensor("sup_local", [SH, out_dim], BF16).ap()
    idxp = nc.dram_tensor("idxp", [128, ng * idxcols], I16).ap()
    secs = [nc.dram_tensor(f"sec{q}", [sec_rows, out_dim], BF16,
                           addr_space="Shared").ap()
            for q in range(4)]

    with tile.TileContext(nc) as tc, tc.tile_pool(name="consts", bufs=1) as consts:
        # replicate the 16-partition index plane to the 128-partition layout
        # dma_gather expects (on device: 8x fewer PCIe bytes than shipping it)
        for g in range(8):
            nc.sync.dma_start(idxp[16 * g:16 * (g + 1), :], idx16[:])

        w_sb = consts.tile([128, kb, out_dim], BF16)
        bp_sb = consts.tile([128, out_dim], F32)
        shift_sb = consts.tile([128, out_dim], F32)
        iota_sb = consts.tile([128, 128], BF16)
        cnt_sb = consts.tile([128, ng], I32)
        acc = consts.tile([128, nt, out_dim], F32)
        gts = consts.tile([128, NGBUF, c_sub, out_dim], BF16)
        for i in range(NGBUF):
            nc.vector.memset(gts[:, i], 0.0)
        for k in range(kb):
            nc.sync.dma_start(w_sb[:, k, :], Wp[k * 128:(k + 1) * 128, :])
        nc.sync.dma_start(bp_sb[:], bp[:])
        nc.sync.dma_start(shift_sb[:], shiftb[:])
        nc.sync.dma_start(iota_sb[:], iota_in[:])
        nc.sync.dma_start(cnt_sb[0:1, :], cnts[:])

        # Phase-2 pools opened first: disjoint SBUF from phase-1 pools, so
        # phase-2 allocations carry no WAR deps on phase-1 releases.
        with (
            tc.tile_pool(name="slabs", bufs=2) as slabs,
            tc.tile_pool(name="sel", bufs=2) as selpool,
            tc.tile_pool(name="p2psum", bufs=6, space="PSUM") as p2psum,
        ):
            with (
                tc.tile_pool(name="xt", bufs=2) as xpool,
                tc.tile_pool(name="p1psum", bufs=2, space="PSUM") as p1psum,
                tc.tile_pool(name="p1out", bufs=4) as p1out,
            ):
                def p1_quarter(q):
                    for st in range(nsup_q):
                        gcol = (q * nsup_q + st) * xcols
                        xt = xpool.tile([128, kb, xcols], BF16)
                        for k in range(kb):
                            nc.sync.dma_start(
                                xt[:, k, :],
                                xT[k * 128:(k + 1) * 128, gcol:gcol + xcols])
                        for j in range(jt):
                            ps = p1psum.tile([128, out_dim], F32)
                            for k in range(kb):
                                nc.tensor.matmul(
                                    ps[:], lhsT=xt[:, k, j * 128:(j + 1) * 128],
                                    rhs=w_sb[:, k, :],
                                    start=(k == 0), stop=(k == kb - 1))
                            so = p1out.tile([128, out_dim], BF16)
                            nc.vector.tensor_tensor(so[:], ps[:], bp_sb[:],
                                                    op=ALU.add)
                            r0 = gcol + j * 128
                            nc.sync.dma_start(sup_local[r0:r0 + 128, :], so[:])

                def allgather(q):
                    nc.gpsimd.collective_compute(
                        "AllGather",
                        ALU.bypass,
                        replica_groups=[list(range(N_CORES))],
                        ins=[sup_local[q * QROWS:(q + 1) * QROWS, :]],
                        outs=[secs[q][:]],
                    )

                nreg = nc.gpsimd.alloc_register("gcnt")
                gbuf_i = 0

                def p2_pass(s):
                    nonlocal gbuf_i
                    for sl in range(nt // tps):
                            idx_sb = slabs.tile([128, tps * idxcols], I16, tag="idx")
                            row_sb = slabs.tile([128, tps * c_sub], BF16, tag="row")
                            val_sb = slabs.tile([128, tps * c_sub], BF16, tag="val")
                            gbase = s * nt + sl * tps
                            nc.sync.dma_start(
                                idx_sb[:], idxp[:, gbase * idxcols:(gbase + tps) * idxcols])
                            nc.sync.dma_start(
                                row_sb[:], rowp[:, gbase * c_sub:(gbase + tps) * c_sub])
                            nc.sync.dma_start(
                                val_sb[:], valp[:, gbase * c_sub:(gbase + tps) * c_sub])
                            for tt in range(tps):
                                t = sl * tps + tt
                                g = s * nt + t
                                # sel[p, c, d] = (row[p, c] == d) * val[p, c]
                                sel = selpool.tile([128, c_sub, 128], BF16)
                                row3 = row_sb[:, tt * c_sub:(tt + 1) * c_sub].unsqueeze(2) \
                                    .to_broadcast([128, c_sub, 128])
                                val3 = val_sb[:, tt * c_sub:(tt + 1) * c_sub].unsqueeze(2) \
                                    .to_broadcast([128, c_sub, 128])
                                iota3 = iota_sb[:].unsqueeze(1) \
                                    .to_broadcast([128, c_sub, 128])
                                nc.vector.tensor_tensor(sel[:], row3, iota3, op=ALU.is_equal)
                                nc.vector.tensor_tensor(sel[:], sel[:], val3, op=ALU.mult)
                                gt = gts[:, gbuf_i % NGBUF]
                                gbuf_i += 1
                                nc.gpsimd.reg_load(nreg, cnt_sb[0:1, g:g + 1])
                                nc.gpsimd.dma_gather(
                                    out_ap=gt[:],
                                    in_ap=secs[s][:],
                                    idxs_ap=idx_sb[:, tt * idxcols:(tt + 1) * idxcols],
                                    num_idxs=nidx,
                                    num_idxs_reg=nreg,
                                    elem_size=out_dim,
                                    single_packet=False,
                                )
                                ps = p2psum.tile([128, out_dim], F32)
                                for u in range(c_sub):
                                    nc.tensor.matmul(
                                        ps[:], lhsT=sel[:, u, :], rhs=gt[:, u, :],
                                        start=(u == 0), stop=(u == c_sub - 1))
                                if s == 0:
                                    nc.vector.tensor_copy(acc[:, t, :], ps[:])
                                elif s < 3:
                                    nc.vector.tensor_tensor(acc[:, t, :], acc[:, t, :],
                                                            ps[:], op=ALU.add)
                                else:
                                    # final section: fuse BN shift + GELU + store
                                    ob = selpool.tile([128, out_dim], F32, tag="ob")
                                    nc.vector.tensor_tensor(ob[:], acc[:, t, :],
                                                            ps[:], op=ALU.add)
                                    ob2 = selpool.tile([128, out_dim], F32, tag="ob2")
                                    nc.vector.tensor_tensor(ob2[:], ob[:],
                                                            shift_sb[:], op=ALU.add)
                                    ob3 = selpool.tile([128, out_dim], BF16, tag="ob3")
                                    nc.scalar.activation(ob3[:], ob2[:], AF.Gelu)
                                    nc.sync.dma_start(out[t * 128:(t + 1) * 128, :],
                                                      ob3[:])

                p1_quarter(0)
                allgather(0)
                p1_quarter(1)
                allgather(1)
                p2_pass(0)
                p1_quarter(2)
                allgather(2)
                p2_pass(1)
                p1_quarter(3)
                allgather(3)
                p2_pass(2)
                p2_pass(3)

    nc.compile()
    return nc


def _preprocess(x, edge_row, edge_col, edge_val, W, b, gamma, beta,
                running_mean, running_var, bn_eps=1e-5):
    n, in_dim = x.shape
    out_dim = W.shape[1]
    assert n == N_CORES * SHARD
    nt = (SHARD + 127) // 128
    nt = ((nt + TPS - 1) // TPS) * TPS
    ng = 4 * nt

    inv_std = 1.0 / np.sqrt(running_var.astype(np.float64) + bn_eps)
    scale = (inv_std * gamma.astype(np.float64)).astype(np.float32)
    shift = (beta.astype(np.float64) - running_mean.astype(np.float64) * inv_std
             * gamma.astype(np.float64)).astype(np.float32)

    xb = x.astype(NPBF16)
    Wp = (W * scale[None, :]).astype(NPBF16)
    bp = np.ascontiguousarray(
        np.broadcast_to((b * scale).astype(np.float32), (128, out_dim)))
    shiftb = np.ascontiguousarray(np.broadcast_to(shift, (128, out_dim)))
    iota = np.ascontiguousarray(
        np.broadcast_to(np.arange(128, dtype=np.float32), (128, 128))).astype(NPBF16)

    per_core = []
    c_sub = 1
    for m in range(N_CORES):
        lo, hi = m * SHARD, (m + 1) * SHARD
        mask = (edge_row >= lo) & (edge_row < hi)
        er = (edge_row[mask] - lo).astype(np.int64)
        ec = edge_col[mask].astype(np.int64)
        ev = edge_val[mask].astype(np.float32)
        src_core = ec // SHARD
        src_r = ec % SHARD
        q = src_r // QROWS
        loc = src_core * QROWS + (src_r - q * QROWS)
        gid = q * nt + (er >> 7)
        order = np.argsort(gid, kind="stable")
        er, ev, loc, gid = er[order], ev[order], loc[order], gid[order]
        counts = np.bincount(gid, minlength=ng)
        per_core.append((er, ev, loc, gid, counts))
        c_sub = max(c_sub, int(((counts + 127) // 128).max()))
    nidx = c_sub * 128
    idxcols = nidx // 16

    in_maps = []
    for m in range(N_CORES):
        er, ev, loc, gid, counts = per_core[m]
        starts = np.zeros(ng, np.int64)
        np.cumsum(counts[:-1], out=starts[1:])
        rank = np.arange(len(er)) - starts[gid]
        rowp = np.zeros((128, ng * c_sub), NPBF16)
        valp = np.zeros((128, ng * c_sub), NPBF16)
        rowp[rank & 127, gid * c_sub + (rank >> 7)] = (er & 127).astype(NPBF16)
        valp[rank & 127, gid * c_sub + (rank >> 7)] = ev.astype(NPBF16)
        idx16 = np.full((16, ng * idxcols), -1, np.int16)
        idx16[rank & 15, gid * idxcols + (rank >> 4)] = loc.astype(np.int16)
        cnts_arr = counts.astype(np.int32)
        empty = np.nonzero(cnts_arr == 0)[0]
        if len(empty):
            idx16[0, empty * idxcols] = 0  # one dummy valid index, val stays 0
            cnts_arr[empty] = 1

        xTm = np.zeros((in_dim, SH), NPBF16)
        xTm[:, :SHARD] = xb[m * SHARD:(m + 1) * SHARD].T
        in_maps.append({
            "xT": np.ascontiguousarray(xTm),
            "Wp": Wp, "bp": bp, "shiftb": shiftb, "iota": iota,
            "idx16": np.ascontiguousarray(idx16),
            "rowp": np.ascontiguousarray(rowp),
            "valp": np.ascontiguousarray(valp),
            "cnts": cnts_arr.reshape(1, ng),
        })

    params = dict(in_dim=in_dim, out_dim=out_dim, nt=nt, c_sub=c_sub,
                  tps=TPS, xcols=XCOLS)
    return in_maps, params, SHARD


def kernel(x, edge_row, edge_col, edge_val, W, b, gamma, beta,
           running_mean, running_var):
    x = np.asarray(x)
    edge_row = np.asarray(edge_row)
    edge_col = np.asarray(edge_col)
    edge_val = np.asarray(edge_val)
    W = np.asarray(W)
    b = np.asarray(b)
    gamma = np.asarray(gamma)
    beta = np.asarray(beta)
    running_mean = np.asarray(running_mean)
    running_var = np.asarray(running_var)

    in_maps, params, shard = _preprocess(
        x, edge_row, edge_col, edge_val, W, b, gamma, beta,
        running_mean, running_var)
    nc = _build_program(**params)
    res = run_bass_kernel_spmd(nc, in_maps, core_ids=list(range(N_CORES)))
    outs = [res.results[m]["out"][:shard].astype(np.float32)
            for m in range(N_CORES)]
    return np.concatenate(outs, axis=0)
